# revision 1
# baseline (speedup 1.0000x reference)
"""Bass/Trainium2 kernel for nn_CrossAttention_33586644254982.

Math: the cross-attention has a single KV token, so softmax over the
key axis (size 1) is exactly 1.0 and the attention output equals V
broadcast over all N query positions. The full module therefore reduces to

    out[b, n, :] = (freq_token[b] @ Wv.T + bv) @ Wo.T + bo     (independent of n)

Q/K projections and spatial_tokens do not affect the output at all.
All device math is exact fp32 (same values as the reference up to
accumulation order).

Strategy: data-parallel over B (16 batches -> 2 per core on 8 cores).
Per core: mm1 computes V = ft @ Wv.T streaming Wv as the PE moving
operand (fewest fp32 instruction pairs), tiny PE transposes give V^T,
mm2 computes O = V @ Wo.T streaming Wo the same way; biases fold into
the PSUM->SBUF copies on DVE. GpSimd partition_broadcast replicates
each O row across 128 partitions, a DVE copy doubles it in the free
dim, and the 24 MiB output shard streams out as 16 large DMAs with
6 KiB descriptors (K_REP consecutive output rows per partition)
alternating between the SP and ACT HWDGE rings. Weight loads are split
across both rings; ft rides SWDGE. Measured ~96-110 us on 8 cores
(variance = HBM-stack sharing between paired NeuronCores during the
store phase), vs ~122 us for the first working version and a ~70 us
pure store-bandwidth floor.
"""

import numpy as np

# Problem shapes (hardcoded per contract - kernel.py is self-contained).
B, N, C, CFD = 16, 4096, 768, 512
N_CORES = 8
BPC = B // N_CORES  # batches per core = 2
P = 128
KA = CFD // P       # k-chunks for mm1 = 4
CC = C // P         # c-chunks = 6
K_REP = 2           # row-replicas materialized in SBUF per DMA burst
T = N // (K_REP * P)  # output DMAs per batch = 8

_CACHE = {}


def _build():
    from concourse import bacc, mybir
    from concourse.tile import TileContext

    f32 = mybir.dt.float32
    nc = bacc.Bacc("TRN2", debug=False, num_devices=N_CORES)

    ftd = nc.dram_tensor("ftd", [P, KA, BPC], f32, kind="ExternalInput").ap()
    WvT = nc.dram_tensor("WvT", [CFD, C], f32, kind="ExternalInput").ap()
    WoT = nc.dram_tensor("WoT", [C, C], f32, kind="ExternalInput").ap()
    bv2 = nc.dram_tensor("bv2", [BPC, C], f32, kind="ExternalInput").ap()
    bo2 = nc.dram_tensor("bo2", [BPC, C], f32, kind="ExternalInput").ap()
    idin = nc.dram_tensor("idin", [BPC, BPC], f32, kind="ExternalInput").ap()
    out = nc.dram_tensor("out", [BPC, N, C], f32, kind="ExternalOutput").ap()

    with TileContext(nc) as tc:
        with (
            tc.tile_pool(name="consts", bufs=1) as consts,
            tc.tile_pool(name="weights", bufs=1) as weights,
            tc.tile_pool(name="small", bufs=1) as small,
            tc.tile_pool(name="repl", bufs=2) as replp,
            tc.tile_pool(name="ps_k", bufs=3, space="PSUM") as ps_k,
            tc.tile_pool(name="ps_t", bufs=4, space="PSUM") as ps_t,
            tc.tile_pool(name="ps_warm", bufs=1, space="PSUM") as ps_warm,
        ):
            # Weight loads split across the two HWDGE rings (SP + ACT) so
            # descriptor generation and transfers run in parallel; Wv first
            # on both rings (it gates mm1), then Wo.
            wv_sb = weights.tile([P, KA, C], f32)
            wv_view = WvT.rearrange("(a p) c -> a p c", p=P)
            wo_sb = weights.tile([P, CC, C], f32)
            wo_view = WoT.rearrange("(m p) c -> m p c", p=P)
            # Wv split by N-halves: mm1's h=0 groups only wait for the
            # first-half transfers (and their completion receipts).
            NS1 = C // 2  # 384
            for h in range(2):
                sl = slice(h * NS1, (h + 1) * NS1)
                nc.sync.dma_start(out=wv_sb[:, 0, sl], in_=wv_view[0][:, sl])
                nc.scalar.dma_start(out=wv_sb[:, 2, sl], in_=wv_view[2][:, sl])
                nc.sync.dma_start(out=wv_sb[:, 1, sl], in_=wv_view[1][:, sl])
                nc.scalar.dma_start(out=wv_sb[:, 3, sl], in_=wv_view[3][:, sl])
            # Tiny constants (3 descriptors each) between Wv and Wo.
            bv_sb = consts.tile([BPC, C], f32)
            nc.scalar.dma_start(out=bv_sb, in_=bv2)
            bo_sb = consts.tile([BPC, C], f32)
            nc.scalar.dma_start(out=bo_sb, in_=bo2)
            ident = consts.tile([BPC, BPC], f32)
            nc.scalar.dma_start(out=ident, in_=idin)
            for m in range(CC):
                (nc.sync if m % 2 == 0 else nc.scalar).dma_start(
                    out=wo_sb[:, m, :], in_=wo_view[m]
                )

            # ft on SWDGE (GpSimd, which is otherwise idle early).
            ft_sb = consts.tile([P, KA, BPC], f32)
            nc.gpsimd.dma_start(out=ft_sb, in_=ftd)

            # Short PE warm-up on zeroed bf16 scratch, sized to end right
            # as Wv lands, so the fp32 chain runs at the warm clock.
            bf16 = mybir.dt.bfloat16
            dum_l = consts.tile([P, P], bf16)
            nc.vector.memset(dum_l, 0.0)
            dum_r = consts.tile([P, 512], bf16)
            nc.vector.memset(dum_r, 0.0)
            ps_w = ps_warm.tile([P, 512], f32)
            for _ in range(6):
                nc.tensor.matmul(ps_w, dum_l, dum_r, start=True, stop=True)

            # mm1: V[b, c] = sum_k ft[b, k] Wv[c, k] + bv[c]
            # Wv streams as the moving operand (N=384): fewest PE
            # instructions for fp32 (each logical matmul = 2 hi/lo passes).
            v_sb = small.tile([BPC, C], f32)
            for h in range(2):
                ps = ps_k.tile([BPC, NS1], f32)
                for a in range(KA):
                    nc.tensor.matmul(
                        ps,
                        ft_sb[:, a, :],
                        wv_sb[:, a, h * NS1 : (h + 1) * NS1],
                        start=(a == 0),
                        stop=(a == KA - 1),
                    )
                nc.vector.tensor_add(
                    v_sb[:, h * NS1 : (h + 1) * NS1],
                    ps,
                    bv_sb[:, h * NS1 : (h + 1) * NS1],
                )

            # PE-transpose V -> VT chunks [128, BPC].
            vt_sb = small.tile([P, CC, BPC], f32)
            for cc in range(CC):
                pst = ps_t.tile([P, BPC], f32)
                nc.tensor.transpose(
                    pst, v_sb[:, cc * P : (cc + 1) * P], ident
                )
                nc.vector.tensor_copy(vt_sb[:, cc, :], pst)

            # mm2: O[b, j] = sum_c V[b, c] Wo[j, c] + bo[j]
            # Wo streams as the moving operand; lhsT = tiny VT columns.
            o_sb = small.tile([BPC, C], f32)
            for h in range(2):
                ps = ps_k.tile([BPC, NS1], f32)
                for m in range(CC):
                    nc.tensor.matmul(
                        ps,
                        vt_sb[:, m, :],
                        wo_sb[:, m, h * NS1 : (h + 1) * NS1],
                        start=(m == 0),
                        stop=(m == CC - 1),
                    )
                nc.vector.tensor_add(
                    o_sb[:, h * NS1 : (h + 1) * NS1],
                    ps,
                    bo_sb[:, h * NS1 : (h + 1) * NS1],
                )

            # O rows at partition 0: b=0 aliases o_sb row 0; b=1 moves to
            # partition 0 via a tiny SBUF->SBUF DMA (DMAs have no
            # partition-base restriction, unlike compute engines). The DMA
            # is emitted inside the b-loop AFTER pb(b=0) so the in-order
            # GpSimd stream broadcasts b=0 as soon as o_sb lands.
            orow1 = small.tile([1, C], f32)
            orow = [o_sb[0:1, :], orow1]

            # Broadcast O rows across partitions, replicate K_REP times in
            # the free dim, and stream the output shard with 12 KiB
            # descriptors (q=K_REP consecutive output rows per partition).
            outv = out.rearrange("b (t p q) c -> b t p (q c)", p=P, q=K_REP)
            engines = [nc.sync, nc.scalar]
            di = 0
            for b in range(BPC):
                r4 = replp.tile([P, K_REP, C], f32)
                nc.gpsimd.partition_broadcast(r4[:, 0, :], orow[b])
                if b + 1 < BPC:
                    nc.gpsimd.dma_start(out=orow1, in_=o_sb[b + 1 : b + 2, :])
                for rep in range(1, K_REP):
                    nc.vector.tensor_copy(r4[:, rep, :], r4[:, 0, :])
                r4_flat = r4.rearrange("p r c -> p (r c)")
                for t in range(T):
                    engines[di % 2].dma_start(out=outv[b, t], in_=r4_flat)
                    di += 1

    nc.compile()
    return nc


def _get_nc():
    if "nc" not in _CACHE:
        _CACHE["nc"] = _build()
    return _CACHE["nc"]


def _install_ntff_hook():
    """Provide antenv.axon_hooks if the image lacks it (profiling only)."""
    import sys
    import types

    try:
        from antenv.axon_hooks import get_axon_ntff_profile_hook  # noqa: F401

        return
    except ImportError:
        pass
    try:
        import antenv
        from trn_agent_boot.trn_boot import _ntff_profile_via_ctypes

        hook = _ntff_profile_via_ctypes("/opt/axon/libaxon_pjrt.so")
        mod = types.ModuleType("antenv.axon_hooks")
        mod.get_axon_ntff_profile_hook = lambda: hook
        mod.set_axon_ntff_profile_hook = lambda h: None
        sys.modules["antenv.axon_hooks"] = mod
        antenv.axon_hooks = mod
    except Exception as e:  # pragma: no cover - profiling is best-effort
        print(f"ntff hook install failed ({e}); tracing disabled", file=sys.stderr)


def _run(inputs, trace=False):
    from concourse import bass_utils

    if trace:
        _install_ntff_hook()
        # Zero-egress container: skip the artifact upload, keep files local.
        bass_utils.upload_artifacts = lambda tmpdir: tmpdir

    nc = _get_nc()
    ft = np.asarray(inputs["freq_token"], np.float32)
    WvT = np.ascontiguousarray(np.asarray(inputs["Wv"], np.float32).T)
    WoT = np.ascontiguousarray(np.asarray(inputs["Wo"], np.float32).T)
    # Bias rows duplicated per batch so DVE tensor_add partitions line up.
    bv2 = np.ascontiguousarray(
        np.broadcast_to(np.asarray(inputs["bv"], np.float32), (BPC, C))
    )
    bo2 = np.ascontiguousarray(
        np.broadcast_to(np.asarray(inputs["bo"], np.float32), (BPC, C))
    )

    in_maps = []
    for i in range(N_CORES):
        ft_loc = ft[BPC * i : BPC * (i + 1)]  # [BPC, CFD]
        # ftd[p, a, b] = ft_loc[b, a*128 + p]
        ftd = np.ascontiguousarray(
            ft_loc.T.reshape(KA, P, BPC).transpose(1, 0, 2)
        )
        in_maps.append(
            {
                "ftd": ftd,
                "WvT": WvT,
                "WoT": WoT,
                "bv2": bv2,
                "bo2": bo2,
                "idin": np.eye(BPC, dtype=np.float32),
            }
        )
    res = bass_utils.run_bass_kernel_spmd(
        nc, in_maps, core_ids=list(range(N_CORES)), trace=trace
    )
    out = np.concatenate([m["out"] for m in res.results], axis=0)
    return out, res


def kernel(**inputs):
    out, _ = _run(inputs, trace=False)
    return out

